# revision 1
# baseline (speedup 1.0000x reference)
"""Llama4-style MoE experts (grouped SwiGLU MLP) on Trainium2, 8 NeuronCores.

Expert-parallel: core i runs expert i's full MLP on its 1024-token slice:
    out = (up * silu(gate)) @ W2,  [gate|up] = h @ W1
Per-core shapes: h [1024, 2048], W1 [2048, 8192], W2 [4096, 2048].

Matmuls run in bf16 on the TensorEngine; f32 inputs are cast on the
VectorEngine.  h is transposed on-chip via PE transpose-mode during the
DMA-bound startup window (free PE time), and the first f-block of W1 is
consumed in narrow half-blocks whose stripes arrive in exactly the
order the PE consumes them, so matmul-1 paces off the DMA stream.

On top of that baseline skeleton:
- The W1 bf16 tile ring holds TWO full f-blocks, and block fb+1's casts
  are emitted between block fb's i-groups, so in steady state the PE
  never waits on W1 casts at block boundaries.
- SwiGLU uses the ScalarEngine's fused Silu (one activation + one
  vector multiply per tile).
- mm2 runs f-outer / tt-inner with all 8 PSUM banks accumulating.  W2
  stripes follow a strict-FIFO DMA+cast schedule half-a-block ahead of
  consumption; the last hb finishes per-tt so the drain staggers into a
  short tail.  Output drains via ScalarEngine copies.
"""

from contextlib import ExitStack

import numpy as np

import concourse.bass as bass
import concourse.mybir as mybir
import concourse.tile as tile
from concourse import bacc
from concourse.bass_utils import run_bass_kernel_spmd
from concourse.masks import make_identity

N_CORES = 8
P = 128
TB = 512  # moving-operand free-dim block (one PSUM bank of f32)

F32 = mybir.dt.float32
BF16 = mybir.dt.bfloat16
ACT_SILU = mybir.ActivationFunctionType.Silu
ACT_COPY = mybir.ActivationFunctionType.Copy

# Per-core problem dims (full problem: 8 experts x 1024 tokens, H=2048, F=4096)
T = 1024
H = 2048
F = 4096


def build_kernel_body(tc, T=T, H=H, F=F):
    nc = tc.nc
    h_d = nc.dram_tensor("hidden_states", [T, H], F32, kind="ExternalInput").ap()
    w1_d = nc.dram_tensor("gate_up_proj", [H, 2 * F], F32, kind="ExternalInput").ap()
    w2_d = nc.dram_tensor("down_proj", [F, H], F32, kind="ExternalInput").ap()
    out_d = nc.dram_tensor("out", [T, H], F32, kind="ExternalOutput").ap()

    n_ht = H // P          # h-tiles (contraction tiles of matmul 1)
    n_ft = F // P          # f-tiles (rows of act; contraction tiles of matmul 2)
    n_tt = T // P          # token tiles (psum partition tiles of matmul 2)
    n_tb = T // TB         # token free-dim blocks in matmul 1
    n_fb = F // TB         # 512-wide f blocks of W1 (per gate/up half)
    n_hb = H // TB         # 512-wide h blocks of W2

    with ExitStack() as ctx:
        const = ctx.enter_context(tc.tile_pool(name="const", bufs=1))
        hcolp = ctx.enter_context(tc.tile_pool(name="hcolp", bufs=6))
        htp = ctx.enter_context(tc.tile_pool(name="htp", bufs=n_ht))
        actp = ctx.enter_context(tc.tile_pool(name="actp", bufs=n_ft))
        wf = ctx.enter_context(tc.tile_pool(name="wf", bufs=6))
        w1b = ctx.enter_context(tc.tile_pool(name="w1b", bufs=192))
        w2fp = ctx.enter_context(tc.tile_pool(name="w2fp", bufs=6))
        b2p = ctx.enter_context(tc.tile_pool(name="b2p", bufs=15))
        silp = ctx.enter_context(tc.tile_pool(name="silp", bufs=2))
        outp = ctx.enter_context(tc.tile_pool(name="outp", bufs=3))
        ps = ctx.enter_context(tc.tile_pool(name="ps", bufs=8, space="PSUM"))

        ident = const.tile([P, P], F32, tag="ident", name="ident")
        make_identity(nc, ident)

        ht = [htp.tile([P, T], BF16, tag="ht", name=f"ht{i}") for i in range(n_ht)]
        act = [actp.tile([P, T], BF16, tag="act", name=f"act{i}") for i in range(n_ft)]

        # ---- W1 streaming ----
        # f32 stripes [128,512] (2KB DMA lines) -> bf16 [128,128] tiles.
        # The bf16 ring holds two full f-blocks (256 tiles), and block
        # fb+1's casts are emitted between block fb's i-groups, so the
        # ring releases/refills per i-column with no boundary race.
        def dma_w1_block(fb, i0=0, ni=None, hh_range=None):
            """Issue DMAs for (part of) one 512-wide f-block."""
            if ni is None:
                ni = TB // P
            if hh_range is None:
                hh_range = range(n_ht)
            w = ni * P
            c0 = fb * TB + i0 * P
            st = {}
            for hh in hh_range:
                sg = wf.tile([P, w], F32, tag="wf", name=f"w1g_{fb}_{hh}_{i0}")
                nc.sync.dma_start(sg[:], w1_d[hh * P : (hh + 1) * P, c0 : c0 + w])
                su = wf.tile([P, w], F32, tag="wf", name=f"w1u_{fb}_{hh}_{i0}")
                nc.sync.dma_start(
                    su[:], w1_d[hh * P : (hh + 1) * P, F + c0 : F + c0 + w]
                )
                st[hh] = (sg, su, i0, ni)
            return st

        def cast_w1_tiles(fb, st, hh_range, wg, wu):
            """Cast staged stripes for hh_range into [128,128] bf16 tiles."""
            for hh in hh_range:
                sg, su, i0, ni = st[hh]
                for k in range(ni):
                    i = i0 + k
                    gb = w1b.tile([P, P], BF16, tag="w1b", name=f"w1gb_{fb}_{hh}_{i}")
                    nc.vector.tensor_copy(out=gb[:], in_=sg[:, k * P : (k + 1) * P])
                    ub = w1b.tile([P, P], BF16, tag="w1b", name=f"w1ub_{fb}_{hh}_{i}")
                    nc.vector.tensor_copy(out=ub[:], in_=su[:, k * P : (k + 1) * P])
                    wg[hh][i] = gb
                    wu[hh][i] = ub

        def swiglu(fi, tb, pg, pu):
            sg = silp.tile([P, TB], BF16, tag="silp", name=f"sig{fi}_{tb}")
            nc.scalar.activation(sg[:], pg[:], ACT_SILU)
            nc.vector.tensor_mul(
                out=act[fi][:, tb * TB : (tb + 1) * TB], in0=pu[:], in1=sg[:]
            )

        def mm1_block(fb, wg, wu, i_range=None, h_outer=False, cast_mid=None):
            """Matmul-1 + SwiGLU for (part of) one 512-wide f-block.

            h_outer orders matmuls h-major with all psum groups live so
            the PE can consume W1 stripes as they arrive (startup).
            cast_mid(i) emits the next block's casts between i-groups.
            """
            if i_range is None:
                i_range = range(TB // P)
            pgs, pus = {}, {}

            def alloc(i):
                fi = fb * (TB // P) + i
                pgs[i] = [
                    ps.tile([P, TB], F32, tag="ps", name=f"pg{fi}_{tb}")
                    for tb in range(n_tb)
                ]
                pus[i] = [
                    ps.tile([P, TB], F32, tag="ps", name=f"pu{fi}_{tb}")
                    for tb in range(n_tb)
                ]

            def mm(i, hh):
                first, last = hh == 0, hh == n_ht - 1
                for p, w in ((pgs[i], wg), (pus[i], wu)):
                    for tb in range(n_tb):
                        nc.tensor.matmul(
                            p[tb][:],
                            lhsT=w[hh][i][:],
                            rhs=ht[hh][:, tb * TB : (tb + 1) * TB],
                            start=first,
                            stop=last,
                        )

            if h_outer:
                for i in i_range:
                    alloc(i)
                for hh in range(n_ht):
                    for i in i_range:
                        mm(i, hh)
                for i in i_range:
                    for tb in range(n_tb):
                        swiglu(fb * (TB // P) + i, tb, pgs[i][tb], pus[i][tb])
            else:
                for i in i_range:
                    alloc(i)
                    for hh in range(n_ht):
                        mm(i, hh)
                    for tb in range(n_tb):
                        swiglu(fb * (TB // P) + i, tb, pgs[i][tb], pus[i][tb])
                    if cast_mid is not None:
                        cast_mid(i)

        # ---- W2 stripes: DMA f32 (SP) + DVE cast into a small ring ----
        b2 = {}

        def w2_stripe(hb, f):
            s = w2fp.tile([P, TB], F32, tag="w2f", name=f"w2f_{hb}_{f}")
            nc.sync.dma_start(
                s[:], w2_d[f * P : (f + 1) * P, hb * TB : (hb + 1) * TB]
            )
            b = b2p.tile([P, TB], BF16, tag="b2", name=f"b2_{hb}_{f}")
            nc.vector.tensor_copy(out=b[:], in_=s[:])
            b2[(hb, f)] = b

        # ---- Phase A: transpose h -> hT bf16 (PE work during DMA ramp) ----
        for ti in range(n_tt):
            hr = hcolp.tile([P, H], F32, tag="hrow", name=f"hrow{ti}", bufs=2)
            nc.scalar.dma_start(hr[:], h_d[ti * P : (ti + 1) * P, :])
            for hh in range(n_ht):
                pt = ps.tile([P, TB], F32, tag="ps", name=f"tp{ti}_{hh}")
                nc.tensor.transpose(pt[:, :P], hr[:, hh * P : (hh + 1) * P], ident)
                nc.vector.tensor_copy(
                    out=ht[hh][:, ti * P : (ti + 1) * P], in_=pt[:, :P]
                )

        # ---- Phase B: G = h @ W1, act = up * silu(gate), stored [f, t] ----
        # fb=0 runs in narrow half-blocks whose W1 stripes arrive in the
        # order the PE consumes them, pacing matmuls off the DMA stream.
        wg = {fb: [[None] * (TB // P) for _ in range(n_ht)] for fb in range(n_fb)}
        wu = {fb: [[None] * (TB // P) for _ in range(n_ht)] for fb in range(n_fb)}
        half = (TB // P) // 2
        st0a = dma_w1_block(0, i0=0, ni=half)
        cast_w1_tiles(0, st0a, range(n_ht), wg[0], wu[0])
        mm1_block(0, wg[0], wu[0], i_range=range(0, half), h_outer=True)
        st0b = dma_w1_block(0, i0=half, ni=(TB // P) - half)
        st1 = dma_w1_block(1)
        cast_w1_tiles(0, st0b, range(n_ht), wg[0], wu[0])
        mm1_block(0, wg[0], wu[0], i_range=range(half, TB // P), h_outer=True)
        # eager cast of block 1 (DVE is idle during fb0's sweeps)
        cast_w1_tiles(1, st1, range(n_ht), wg[1], wu[1])

        st = {1: st1}

        def make_cast_mid(fb_next):
            n_cm = n_ht // (TB // P)

            def cast_mid(i):
                if fb_next >= n_fb:
                    return
                cast_w1_tiles(
                    fb_next, st[fb_next],
                    range(i * n_cm, (i + 1) * n_cm),
                    wg[fb_next], wu[fb_next],
                )
            return cast_mid

        for fb in range(1, n_fb):
            if fb + 1 < n_fb:
                st[fb + 1] = dma_w1_block(fb + 1)
            mm1_block(fb, wg[fb], wu[fb], cast_mid=make_cast_mid(fb + 1))
            if fb == n_fb - 2:
                # W2 hb0 prefetch (after the final W1 casts on the DVE
                # queue; capped at the b2 ring depth)
                for f in range(14):
                    w2_stripe(0, f)

        # ---- Phase C: out = act @ W2, f-outer / tt-inner, 8 psum banks ----
        def drain(hb, tt, po):
            ob = outp.tile([P, TB], F32, tag="outp", name=f"ob{hb}_{tt}")
            nc.scalar.activation(ob[:], po[:], ACT_COPY)
            nc.scalar.dma_start(
                out_d[tt * P : (tt + 1) * P, hb * TB : (hb + 1) * TB], ob[:]
            )

        # strict FIFO stripe schedule: allocation runs exactly AHEAD of
        # consumption so the b2 ring can never form a cross-phase cycle
        SCHED = [(hb, f) for hb in range(n_hb) for f in range(n_ft)]
        AHEAD = 14

        def trickle(g):
            if g + AHEAD < len(SCHED):
                w2_stripe(*SCHED[g + AHEAD])

        FTAIL = 4  # last-hb f-tiles processed per-tt so the drain staggers
        for hb in range(n_hb):
            last_hb = hb == n_hb - 1
            po = [ps.tile([P, TB], F32, tag="ps", name=f"po{hb}_{tt}")
                  for tt in range(n_tt)]
            n_f_main = n_ft - FTAIL if last_hb else n_ft
            for f in range(n_f_main):
                trickle(hb * n_ft + f)
                rhs = b2[(hb, f)][:]
                for tt in range(n_tt):
                    nc.tensor.matmul(
                        po[tt][:],
                        lhsT=act[f][:, tt * P : (tt + 1) * P],
                        rhs=rhs,
                        start=(f == 0),
                        stop=(not last_hb and f == n_ft - 1),
                    )
            if not last_hb:
                for tt in range(n_tt):
                    drain(hb, tt, po[tt])
            else:
                for f in range(n_f_main, n_ft):
                    trickle(hb * n_ft + f)
                for tt in range(n_tt):
                    for f in range(n_f_main, n_ft):
                        nc.tensor.matmul(
                            po[tt][:],
                            lhsT=act[f][:, tt * P : (tt + 1) * P],
                            rhs=b2[(hb, f)][:],
                            start=False,
                            stop=(f == n_ft - 1),
                        )
                    drain(hb, tt, po[tt])


def build_nc(T=T, H=H, F=F):
    nc = bacc.Bacc(
        "TRN2", target_bir_lowering=False, debug=False, enable_asserts=False
    )
    with tile.TileContext(nc) as tc:
        build_kernel_body(tc, T=T, H=H, F=F)
    nc.compile()
    return nc


_NC_CACHE = None


def run(hidden_states, gate_up_proj, down_proj, trace=False, **kw):
    """Run on the 8 NeuronCores; returns (output, BassKernelResults)."""
    global _NC_CACHE
    if _NC_CACHE is None:
        _NC_CACHE = build_nc()
    nc = _NC_CACHE

    hs = np.ascontiguousarray(np.asarray(hidden_states), dtype=np.float32)
    gup = np.ascontiguousarray(np.asarray(gate_up_proj), dtype=np.float32)
    dp = np.ascontiguousarray(np.asarray(down_proj), dtype=np.float32)
    assert hs.shape == (N_CORES * T, H), hs.shape
    assert gup.shape == (N_CORES, H, 2 * F), gup.shape
    assert dp.shape == (N_CORES, F, H), dp.shape

    in_maps = [
        {
            "hidden_states": np.ascontiguousarray(hs[i * T : (i + 1) * T]),
            "gate_up_proj": np.ascontiguousarray(gup[i]),
            "down_proj": np.ascontiguousarray(dp[i]),
        }
        for i in range(N_CORES)
    ]
    res = run_bass_kernel_spmd(
        nc, in_maps, core_ids=list(range(N_CORES)), trace=trace, **kw
    )
    out = np.concatenate(
        [res.results[i]["out"] for i in range(N_CORES)], axis=0
    ).astype(np.float32)
    return out, res


def kernel(hidden_states, gate_up_proj, down_proj):
    out, _ = run(hidden_states, gate_up_proj, down_proj, trace=False)
    return out



# revision 5
# speedup vs baseline: 1.0128x; 1.0128x over previous
"""Llama4-style MoE experts (grouped SwiGLU MLP) on Trainium2, 8 NeuronCores.

Expert-parallel: core i runs expert i's full MLP on its 1024-token slice:
    out = (up * silu(gate)) @ W2,  [gate|up] = h @ W1
Per-core shapes: h [1024, 2048], W1 [2048, 8192], W2 [4096, 2048].

Matmuls run in bf16 on the TensorEngine.  All weight traffic uses gpsimd
SWDGE *casting* DMAs (f32 HBM -> bf16 SBUF directly), so there is no
f32 staging and no DVE cast pipeline: the PE waits only on DMA-complete
semaphores.  The W1 bf16 stripe ring holds two full 512-wide f-blocks,
so block fb+1 streams in while fb computes with zero boundary stalls.

h loads are split across the sync and vector HWDGE queues (4 rows
each); rows are PE-transposed (f32, via identity) as they land, and
fb0's matmuls run tb-major with i-pair PSUM groups so mm1 starts once
the first 4 rows have landed, with rows 4-7 transposed in the gaps.

mm2 runs f-outer / tt-inner with all 8 PSUM banks accumulating; W2
stripes trickle in ~12 f-tiles ahead of consumption.  Output drains
alternate the Scalar and Vector engines (copy + per-engine DMA queue),
and the last h-block's f-tail is processed per-tt so the drain
staggers into a short tail.
"""

from contextlib import ExitStack

import numpy as np

import concourse.bass as bass
import concourse.mybir as mybir
import concourse.tile as tile
from concourse import bacc
from concourse.bass_utils import run_bass_kernel_spmd
from concourse.masks import make_identity

N_CORES = 8
P = 128
TB = 512  # moving-operand free-dim block (one PSUM bank of f32)

F32 = mybir.dt.float32
BF16 = mybir.dt.bfloat16
ACT_SILU = mybir.ActivationFunctionType.Silu
ACT_COPY = mybir.ActivationFunctionType.Copy

# Per-core problem dims (full problem: 8 experts x 1024 tokens, H=2048, F=4096)
T = 1024
H = 2048
F = 4096


def build_kernel_body(tc, T=T, H=H, F=F):
    nc = tc.nc
    h_d = nc.dram_tensor("hidden_states", [T, H], F32, kind="ExternalInput").ap()
    w1_d = nc.dram_tensor("gate_up_proj", [H, 2 * F], F32, kind="ExternalInput").ap()
    w2_d = nc.dram_tensor("down_proj", [F, H], F32, kind="ExternalInput").ap()
    out_d = nc.dram_tensor("out", [T, H], F32, kind="ExternalOutput").ap()

    n_ht = H // P          # h-tiles (contraction tiles of matmul 1)
    n_ft = F // P          # f-tiles (rows of act; contraction tiles of matmul 2)
    n_tt = T // P          # token tiles (psum partition tiles of matmul 2)
    n_tb = T // TB         # token free-dim blocks in matmul 1
    n_fb = F // TB         # 512-wide f blocks of W1 (per gate/up half)
    n_hb = H // TB         # 512-wide h blocks of W2

    with ExitStack() as ctx:
        const = ctx.enter_context(tc.tile_pool(name="const", bufs=1))
        hcolp = ctx.enter_context(tc.tile_pool(name="hcolp", bufs=2))
        htp = ctx.enter_context(tc.tile_pool(name="htp", bufs=n_ht))
        actp = ctx.enter_context(tc.tile_pool(name="actp", bufs=n_ft))
        w1p = ctx.enter_context(tc.tile_pool(name="w1p", bufs=64))
        b2p = ctx.enter_context(tc.tile_pool(name="b2p", bufs=14))
        silp = ctx.enter_context(tc.tile_pool(name="silp", bufs=4))
        outp = ctx.enter_context(tc.tile_pool(name="outp", bufs=5))
        ps = ctx.enter_context(tc.tile_pool(name="ps", bufs=8, space="PSUM"))

        ident = const.tile([P, P], F32, tag="ident", name="ident")
        make_identity(nc, ident)

        ht = [htp.tile([P, T], BF16, tag="ht", name=f"ht{i}") for i in range(n_ht)]
        act = [actp.tile([P, T], BF16, tag="act", name=f"act{i}") for i in range(n_ft)]

        # ---- h rows: two HWDGE queues (sync even rows, vector odd rows) ----
        hrow = {}

        def dma_h_row(ti):
            hr = hcolp.tile([P, H], F32, tag="hrow", name=f"hrow{ti}", bufs=2)
            eng = nc.sync if ti % 2 == 0 else nc.scalar
            eng.dma_start(hr[:], h_d[ti * P : (ti + 1) * P, :])
            hrow[ti] = hr

        def transpose_row(ti):
            """PE-transpose one 128-token row block into ht (f32 -> bf16)."""
            hr = hrow[ti]
            for hh in range(n_ht):
                pt = ps.tile([P, TB], F32, tag="ps", name=f"tp{ti}_{hh}")
                nc.tensor.transpose(pt[:, :P], hr[:, hh * P : (hh + 1) * P], ident)
                nc.vector.tensor_copy(
                    out=ht[hh][:, ti * P : (ti + 1) * P], in_=pt[:, :P]
                )

        # ---- W1: gpsimd casting DMAs, f32 HBM -> bf16 SBUF stripes ----
        w1g, w1u = {}, {}

        def dma_w1_block(fb):
            c0 = fb * TB
            for hh in range(n_ht):
                g = w1p.tile([P, TB], BF16, tag="w1", name=f"w1g_{fb}_{hh}")
                nc.gpsimd.dma_start(
                    out=g[:], in_=w1_d[hh * P : (hh + 1) * P, c0 : c0 + TB]
                )
                u = w1p.tile([P, TB], BF16, tag="w1", name=f"w1u_{fb}_{hh}")
                nc.gpsimd.dma_start(
                    out=u[:], in_=w1_d[hh * P : (hh + 1) * P, F + c0 : F + c0 + TB]
                )
                w1g[(fb, hh)] = g
                w1u[(fb, hh)] = u

        def swiglu(fi, tb, pg, pu):
            sg = silp.tile([P, TB], BF16, tag="silp", name=f"sig{fi}_{tb}")
            nc.scalar.activation(sg[:], pg[:], ACT_SILU)
            nc.vector.tensor_mul(
                out=act[fi][:, tb * TB : (tb + 1) * TB], in0=pu[:], in1=sg[:]
            )

        # ---- W2: gpsimd casting DMAs straight into the b2 ring ----
        b2 = {}

        def w2_stripe(hb, f):
            b = b2p.tile([P, TB], BF16, tag="b2", name=f"b2_{hb}_{f}")
            nc.gpsimd.dma_start(
                out=b[:], in_=w2_d[f * P : (f + 1) * P, hb * TB : (hb + 1) * TB]
            )
            b2[(hb, f)] = b

        # ---- Phase A/B0: h loads + transposes interleaved with fb0 ----
        for ti in range(8):
            dma_h_row(ti)
        dma_w1_block(0)
        dma_w1_block(1)
        for ti in range(4):
            transpose_row(ti)

        def sweep_fb0(tb, iset):
            """h-outer sweep over one tb-half of fb0 for an i-pair.

            Paces off the W1 DMA stream: matmul (i, hh) waits only on the
            gpsimd casting DMA of stripe (0, hh).
            """
            pg = {i: ps.tile([P, TB], F32, tag="ps", name=f"pg0_{i}_{tb}")
                  for i in iset}
            pu = {i: ps.tile([P, TB], F32, tag="ps", name=f"pu0_{i}_{tb}")
                  for i in iset}
            for hh in range(n_ht):
                first, last = hh == 0, hh == n_ht - 1
                for i in iset:
                    for p, w in ((pg, w1g), (pu, w1u)):
                        nc.tensor.matmul(
                            p[i][:],
                            lhsT=w[(0, hh)][:, i * P : (i + 1) * P],
                            rhs=ht[hh][:, tb * TB : (tb + 1) * TB],
                            start=first,
                            stop=last,
                        )
            for i in iset:
                swiglu(i, tb, pg[i], pu[i])

        # rows 4-7 transpose between sweeps (they land while sweep A runs;
        # no psum accumulation groups are held at these points)
        sweep_fb0(0, (0, 1))
        transpose_row(4)
        transpose_row(5)
        sweep_fb0(0, (2, 3))
        transpose_row(6)
        transpose_row(7)
        sweep_fb0(1, (0, 1))
        sweep_fb0(1, (2, 3))

        # ---- Phase B: steady-state f-blocks 1..n_fb-1 ----
        for fb in range(1, n_fb):
            if fb + 1 < n_fb:
                dma_w1_block(fb + 1)
            for i in range(TB // P):
                fi = fb * (TB // P) + i
                pg = {tb: ps.tile([P, TB], F32, tag="ps", name=f"pg{fi}_{tb}")
                      for tb in range(n_tb)}
                pu = {tb: ps.tile([P, TB], F32, tag="ps", name=f"pu{fi}_{tb}")
                      for tb in range(n_tb)}
                for hh in range(n_ht):
                    first, last = hh == 0, hh == n_ht - 1
                    for p, w in ((pg, w1g), (pu, w1u)):
                        for tb in range(n_tb):
                            nc.tensor.matmul(
                                p[tb][:],
                                lhsT=w[(fb, hh)][:, i * P : (i + 1) * P],
                                rhs=ht[hh][:, tb * TB : (tb + 1) * TB],
                                start=first,
                                stop=last,
                            )
                for tb in range(n_tb):
                    swiglu(fi, tb, pg[tb], pu[tb])
            if fb == n_fb - 2:
                # W2 hb0 prefetch (capped at the b2 ring depth)
                for f in range(12):
                    w2_stripe(0, f)

        # ---- Phase C: out = act @ W2, f-outer / tt-inner, 8 psum banks ----
        def drain(hb, tt, po):
            ob = outp.tile([P, TB], F32, tag="outp", name=f"ob{hb}_{tt}")
            if tt % 2 == 0:
                nc.scalar.activation(ob[:], po[:], ACT_COPY)
                nc.scalar.dma_start(
                    out_d[tt * P : (tt + 1) * P, hb * TB : (hb + 1) * TB], ob[:]
                )
            else:
                nc.vector.tensor_copy(out=ob[:], in_=po[:])
                nc.sync.dma_start(
                    out_d[tt * P : (tt + 1) * P, hb * TB : (hb + 1) * TB], ob[:]
                )

        # strict FIFO stripe schedule: allocation runs exactly AHEAD of
        # consumption so the b2 ring can never form a cross-phase cycle
        SCHED = [(hb, f) for hb in range(n_hb) for f in range(n_ft)]
        AHEAD = 12

        def trickle(g):
            if g + AHEAD < len(SCHED):
                w2_stripe(*SCHED[g + AHEAD])

        FTAIL = 4  # last-hb f-tiles processed per-tt so the drain staggers
        for hb in range(n_hb):
            last_hb = hb == n_hb - 1
            po = [ps.tile([P, TB], F32, tag="ps", name=f"po{hb}_{tt}")
                  for tt in range(n_tt)]
            n_f_main = n_ft - FTAIL if last_hb else n_ft
            for f in range(n_f_main):
                trickle(hb * n_ft + f)
                rhs = b2[(hb, f)][:]
                for tt in range(n_tt):
                    nc.tensor.matmul(
                        po[tt][:],
                        lhsT=act[f][:, tt * P : (tt + 1) * P],
                        rhs=rhs,
                        start=(f == 0),
                        stop=(not last_hb and f == n_ft - 1),
                    )
            if not last_hb:
                for tt in range(n_tt):
                    drain(hb, tt, po[tt])
            else:
                for f in range(n_f_main, n_ft):
                    trickle(hb * n_ft + f)
                for tt in range(n_tt):
                    for f in range(n_f_main, n_ft):
                        nc.tensor.matmul(
                            po[tt][:],
                            lhsT=act[f][:, tt * P : (tt + 1) * P],
                            rhs=b2[(hb, f)][:],
                            start=False,
                            stop=(f == n_ft - 1),
                        )
                    drain(hb, tt, po[tt])


def build_nc(T=T, H=H, F=F):
    nc = bacc.Bacc(
        "TRN2", target_bir_lowering=False, debug=False, enable_asserts=False
    )
    with tile.TileContext(nc) as tc:
        build_kernel_body(tc, T=T, H=H, F=F)
    nc.compile()
    return nc


_NC_CACHE = None


def run(hidden_states, gate_up_proj, down_proj, trace=False, **kw):
    """Run on the 8 NeuronCores; returns (output, BassKernelResults)."""
    global _NC_CACHE
    if _NC_CACHE is None:
        _NC_CACHE = build_nc()
    nc = _NC_CACHE

    hs = np.ascontiguousarray(np.asarray(hidden_states), dtype=np.float32)
    gup = np.ascontiguousarray(np.asarray(gate_up_proj), dtype=np.float32)
    dp = np.ascontiguousarray(np.asarray(down_proj), dtype=np.float32)
    assert hs.shape == (N_CORES * T, H), hs.shape
    assert gup.shape == (N_CORES, H, 2 * F), gup.shape
    assert dp.shape == (N_CORES, F, H), dp.shape

    in_maps = [
        {
            "hidden_states": np.ascontiguousarray(hs[i * T : (i + 1) * T]),
            "gate_up_proj": np.ascontiguousarray(gup[i]),
            "down_proj": np.ascontiguousarray(dp[i]),
        }
        for i in range(N_CORES)
    ]
    res = run_bass_kernel_spmd(
        nc, in_maps, core_ids=list(range(N_CORES)), trace=trace, **kw
    )
    out = np.concatenate(
        [res.results[i]["out"] for i in range(N_CORES)], axis=0
    ).astype(np.float32)
    return out, res


def kernel(hidden_states, gate_up_proj, down_proj):
    out, _ = run(hidden_states, gate_up_proj, down_proj, trace=False)
    return out


# revision 9
# speedup vs baseline: 1.0320x; 1.0190x over previous
"""Llama4-style MoE experts (grouped SwiGLU MLP) on Trainium2, 8 NeuronCores.

Expert-parallel: core i runs expert i's full MLP on its 1024-token slice:
    out = (up * silu(gate)) @ W2,  [gate|up] = h @ W1
Per-core shapes: h [1024, 2048], W1 [2048, 8192], W2 [4096, 2048].

Matmuls run in bf16 on the TensorEngine.  All weight traffic uses gpsimd
SWDGE *casting* DMAs (f32 HBM -> bf16 SBUF directly), so there is no
f32 staging and no DVE cast pipeline: the PE waits only on DMA-complete
semaphores.  The W1 bf16 stripe ring holds two full 512-wide f-blocks,
so block fb+1 streams in while fb computes with zero boundary stalls.

h loads are split across the sync and vector HWDGE queues (4 rows
each); rows are PE-transposed (f32, via identity) as they land, and
fb0's matmuls run tb-major with i-pair PSUM groups so mm1 starts once
the first 4 rows have landed, with rows 4-7 transposed in the gaps.

mm2 runs f-outer / tt-inner with all 8 PSUM banks accumulating; W2
stripes trickle in ~12 f-tiles ahead of consumption.  Output drains
alternate the Scalar and Vector engines (copy + per-engine DMA queue),
and the last h-block's f-tail is processed per-tt so the drain
staggers into a short tail.
"""

from contextlib import ExitStack

import numpy as np

import concourse.bass as bass
import concourse.mybir as mybir
import concourse.tile as tile
from concourse import bacc
from concourse.bass_utils import run_bass_kernel_spmd
from concourse.masks import make_identity

N_CORES = 8
P = 128
TB = 512  # moving-operand free-dim block (one PSUM bank of f32)

F32 = mybir.dt.float32
BF16 = mybir.dt.bfloat16
ACT_SILU = mybir.ActivationFunctionType.Silu
ACT_COPY = mybir.ActivationFunctionType.Copy

# Per-core problem dims (full problem: 8 experts x 1024 tokens, H=2048, F=4096)
T = 1024
H = 2048
F = 4096


def build_kernel_body(tc, T=T, H=H, F=F):
    nc = tc.nc
    h_d = nc.dram_tensor("hidden_states", [T, H], F32, kind="ExternalInput").ap()
    w1_d = nc.dram_tensor("gate_up_proj", [H, 2 * F], F32, kind="ExternalInput").ap()
    w2_d = nc.dram_tensor("down_proj", [F, H], F32, kind="ExternalInput").ap()
    out_d = nc.dram_tensor("out", [T, H], F32, kind="ExternalOutput").ap()

    n_ht = H // P          # h-tiles (contraction tiles of matmul 1)
    n_ft = F // P          # f-tiles (rows of act; contraction tiles of matmul 2)
    n_tt = T // P          # token tiles (psum partition tiles of matmul 2)
    n_tb = T // TB         # token free-dim blocks in matmul 1
    n_fb = F // TB         # 512-wide f blocks of W1 (per gate/up half)
    n_hb = H // TB         # 512-wide h blocks of W2

    with ExitStack() as ctx:
        const = ctx.enter_context(tc.tile_pool(name="const", bufs=1))
        hcolp = ctx.enter_context(tc.tile_pool(name="hcolp", bufs=2))
        htp = ctx.enter_context(tc.tile_pool(name="htp", bufs=n_ht))
        actp = ctx.enter_context(tc.tile_pool(name="actp", bufs=n_ft))
        w1p = ctx.enter_context(tc.tile_pool(name="w1p", bufs=62))
        b2p = ctx.enter_context(tc.tile_pool(name="b2p", bufs=12))
        silp = ctx.enter_context(tc.tile_pool(name="silp", bufs=2))
        outp = ctx.enter_context(tc.tile_pool(name="outp", bufs=8))
        ps = ctx.enter_context(tc.tile_pool(name="ps", bufs=8, space="PSUM"))

        ident = const.tile([P, P], F32, tag="ident", name="ident")
        make_identity(nc, ident)

        ht = [htp.tile([P, T], BF16, tag="ht", name=f"ht{i}") for i in range(n_ht)]
        act = [actp.tile([P, T], BF16, tag="act", name=f"act{i}") for i in range(n_ft)]

        # ---- h rows: two HWDGE queues (sync even rows, vector odd rows) ----
        hrow = {}

        def dma_h_row(ti, split=False):
            hr = hcolp.tile([P, H], F32, tag="hrow", name=f"hrow{ti}", bufs=2)
            if split:
                # halves on both HWDGE queues so early rows land sooner
                nc.sync.dma_start(
                    hr[:, : H // 2], h_d[ti * P : (ti + 1) * P, : H // 2]
                )
                nc.scalar.dma_start(
                    hr[:, H // 2 :], h_d[ti * P : (ti + 1) * P, H // 2 :]
                )
            else:
                eng = nc.sync if ti % 2 == 0 else nc.scalar
                eng.dma_start(hr[:], h_d[ti * P : (ti + 1) * P, :])
            hrow[ti] = hr

        def transpose_row(ti):
            """PE-transpose one 128-token row block into ht (f32 -> bf16)."""
            hr = hrow[ti]
            for hh in range(n_ht):
                pt = ps.tile([P, TB], F32, tag="ps", name=f"tp{ti}_{hh}")
                nc.tensor.transpose(pt[:, :P], hr[:, hh * P : (hh + 1) * P], ident)
                nc.vector.tensor_copy(
                    out=ht[hh][:, ti * P : (ti + 1) * P], in_=pt[:, :P]
                )

        # ---- W1: gpsimd casting DMAs, f32 HBM -> bf16 SBUF stripes ----
        w1g, w1u = {}, {}

        def dma_w1_block(fb):
            c0 = fb * TB
            for hh in range(n_ht):
                g = w1p.tile([P, TB], BF16, tag="w1", name=f"w1g_{fb}_{hh}")
                nc.gpsimd.dma_start(
                    out=g[:], in_=w1_d[hh * P : (hh + 1) * P, c0 : c0 + TB]
                )
                u = w1p.tile([P, TB], BF16, tag="w1", name=f"w1u_{fb}_{hh}")
                nc.gpsimd.dma_start(
                    out=u[:], in_=w1_d[hh * P : (hh + 1) * P, F + c0 : F + c0 + TB]
                )
                w1g[(fb, hh)] = g
                w1u[(fb, hh)] = u

        def swiglu(fi, tb, pg, pu):
            sg = silp.tile([P, TB], BF16, tag="silp", name=f"sig{fi}_{tb}")
            nc.scalar.activation(sg[:], pg[:], ACT_SILU)
            nc.vector.tensor_mul(
                out=act[fi][:, tb * TB : (tb + 1) * TB], in0=pu[:], in1=sg[:]
            )

        # ---- W2: gpsimd casting DMAs straight into the b2 ring ----
        b2 = {}

        def w2_stripe(hb, f):
            b = b2p.tile([P, TB], BF16, tag="b2", name=f"b2_{hb}_{f}")
            nc.gpsimd.dma_start(
                out=b[:], in_=w2_d[f * P : (f + 1) * P, hb * TB : (hb + 1) * TB]
            )
            b2[(hb, f)] = b

        # ---- Phase A/B0: h loads + transposes interleaved with fb0 ----
        for ti in range(4):
            dma_h_row(ti, split=True)
        dma_w1_block(0)
        for ti in range(4, 8):
            dma_h_row(ti)
        for ti in range(4):
            transpose_row(ti)

        def sweep_fb0(tb, iset):
            """h-outer sweep over one tb-half of fb0 for an i-pair.

            Paces off the W1 DMA stream: matmul (i, hh) waits only on the
            gpsimd casting DMA of stripe (0, hh).
            """
            pg = {i: ps.tile([P, TB], F32, tag="ps", name=f"pg0_{i}_{tb}")
                  for i in iset}
            pu = {i: ps.tile([P, TB], F32, tag="ps", name=f"pu0_{i}_{tb}")
                  for i in iset}
            for hh in range(n_ht):
                first, last = hh == 0, hh == n_ht - 1
                for i in iset:
                    for p, w in ((pg, w1g), (pu, w1u)):
                        nc.tensor.matmul(
                            p[i][:],
                            lhsT=w[(0, hh)][:, i * P : (i + 1) * P],
                            rhs=ht[hh][:, tb * TB : (tb + 1) * TB],
                            start=first,
                            stop=last,
                        )
            for i in iset:
                swiglu(i, tb, pg[i], pu[i])

        # rows 4-7 transpose between sweeps (they land while sweep A runs;
        # no psum accumulation groups are held at these points).  fb1's
        # stripes are issued after sweep A so startup DMA bandwidth goes
        # to h and fb0 first.
        sweep_fb0(0, (0, 1))
        dma_w1_block(1)
        transpose_row(4)
        transpose_row(5)
        sweep_fb0(0, (2, 3))
        transpose_row(6)
        transpose_row(7)
        sweep_fb0(1, (0, 1))
        sweep_fb0(1, (2, 3))

        # ---- Phase B: steady-state f-blocks 1..n_fb-1 ----
        for fb in range(1, n_fb):
            if fb + 1 < n_fb:
                dma_w1_block(fb + 1)
            for i in range(TB // P):
                fi = fb * (TB // P) + i
                pg = {tb: ps.tile([P, TB], F32, tag="ps", name=f"pg{fi}_{tb}")
                      for tb in range(n_tb)}
                pu = {tb: ps.tile([P, TB], F32, tag="ps", name=f"pu{fi}_{tb}")
                      for tb in range(n_tb)}
                for hh in range(n_ht):
                    first, last = hh == 0, hh == n_ht - 1
                    for p, w in ((pg, w1g), (pu, w1u)):
                        for tb in range(n_tb):
                            nc.tensor.matmul(
                                p[tb][:],
                                lhsT=w[(fb, hh)][:, i * P : (i + 1) * P],
                                rhs=ht[hh][:, tb * TB : (tb + 1) * TB],
                                start=first,
                                stop=last,
                            )
                for tb in range(n_tb):
                    swiglu(fi, tb, pg[tb], pu[tb])
            if fb == n_fb - 2:
                # W2 hb0 prefetch (capped at the b2 ring depth)
                for f in range(12):
                    w2_stripe(0, f)

        # ---- Phase C: out = act @ W2, f-outer / tt-inner, 8 psum banks ----
        def drain(hb, tt, po):
            ob = outp.tile([P, TB], F32, tag="outp", name=f"ob{hb}_{tt}")
            if tt % 2 == 0:
                nc.scalar.activation(ob[:], po[:], ACT_COPY)
                nc.scalar.dma_start(
                    out_d[tt * P : (tt + 1) * P, hb * TB : (hb + 1) * TB], ob[:]
                )
            else:
                nc.vector.tensor_copy(out=ob[:], in_=po[:])
                nc.sync.dma_start(
                    out_d[tt * P : (tt + 1) * P, hb * TB : (hb + 1) * TB], ob[:]
                )

        # strict FIFO stripe schedule: allocation runs exactly AHEAD of
        # consumption so the b2 ring can never form a cross-phase cycle
        SCHED = [(hb, f) for hb in range(n_hb) for f in range(n_ft)]
        AHEAD = 12

        def trickle(g):
            if g + AHEAD < len(SCHED):
                w2_stripe(*SCHED[g + AHEAD])

        FTAIL = 4  # last-hb f-tiles processed per-tt so the drain staggers
        for hb in range(n_hb):
            last_hb = hb == n_hb - 1
            po = [ps.tile([P, TB], F32, tag="ps", name=f"po{hb}_{tt}")
                  for tt in range(n_tt)]
            n_f_main = n_ft - FTAIL if last_hb else n_ft
            for f in range(n_f_main):
                trickle(hb * n_ft + f)
                rhs = b2[(hb, f)][:]
                for tt in range(n_tt):
                    nc.tensor.matmul(
                        po[tt][:],
                        lhsT=act[f][:, tt * P : (tt + 1) * P],
                        rhs=rhs,
                        start=(f == 0),
                        stop=(not last_hb and f == n_ft - 1),
                    )
            if not last_hb:
                for tt in range(n_tt):
                    drain(hb, tt, po[tt])
            else:
                for f in range(n_f_main, n_ft):
                    trickle(hb * n_ft + f)
                for tt in range(n_tt):
                    for f in range(n_f_main, n_ft):
                        nc.tensor.matmul(
                            po[tt][:],
                            lhsT=act[f][:, tt * P : (tt + 1) * P],
                            rhs=b2[(hb, f)][:],
                            start=False,
                            stop=(f == n_ft - 1),
                        )
                    drain(hb, tt, po[tt])


def build_nc(T=T, H=H, F=F):
    nc = bacc.Bacc(
        "TRN2", target_bir_lowering=False, debug=False, enable_asserts=False
    )
    with tile.TileContext(nc) as tc:
        build_kernel_body(tc, T=T, H=H, F=F)
    nc.compile()
    return nc


_NC_CACHE = None


def run(hidden_states, gate_up_proj, down_proj, trace=False, **kw):
    """Run on the 8 NeuronCores; returns (output, BassKernelResults)."""
    global _NC_CACHE
    if _NC_CACHE is None:
        _NC_CACHE = build_nc()
    nc = _NC_CACHE

    hs = np.ascontiguousarray(np.asarray(hidden_states), dtype=np.float32)
    gup = np.ascontiguousarray(np.asarray(gate_up_proj), dtype=np.float32)
    dp = np.ascontiguousarray(np.asarray(down_proj), dtype=np.float32)
    assert hs.shape == (N_CORES * T, H), hs.shape
    assert gup.shape == (N_CORES, H, 2 * F), gup.shape
    assert dp.shape == (N_CORES, F, H), dp.shape

    in_maps = [
        {
            "hidden_states": np.ascontiguousarray(hs[i * T : (i + 1) * T]),
            "gate_up_proj": np.ascontiguousarray(gup[i]),
            "down_proj": np.ascontiguousarray(dp[i]),
        }
        for i in range(N_CORES)
    ]
    res = run_bass_kernel_spmd(
        nc, in_maps, core_ids=list(range(N_CORES)), trace=trace, **kw
    )
    out = np.concatenate(
        [res.results[i]["out"] for i in range(N_CORES)], axis=0
    ).astype(np.float32)
    return out, res


def kernel(hidden_states, gate_up_proj, down_proj):
    out, _ = run(hidden_states, gate_up_proj, down_proj, trace=False)
    return out


# revision 14
# speedup vs baseline: 1.0332x; 1.0011x over previous
"""Llama4-style MoE experts (grouped SwiGLU MLP) on Trainium2, 8 NeuronCores.

Expert-parallel: core i runs expert i's full MLP on its 1024-token slice:
    out = (up * silu(gate)) @ W2,  [gate|up] = h @ W1
Per-core shapes: h [1024, 2048], W1 [2048, 8192], W2 [4096, 2048].

Matmuls run in bf16 on the TensorEngine.  All weight traffic uses gpsimd
SWDGE *casting* DMAs (f32 HBM -> bf16 SBUF directly), so there is no
f32 staging and no DVE cast pipeline: the PE waits only on DMA-complete
semaphores.  The W1 bf16 stripe ring holds two full 512-wide f-blocks,
so block fb+1 streams in while fb computes with zero boundary stalls.

h loads are split across the sync and vector HWDGE queues (4 rows
each); rows are PE-transposed (f32, via identity) as they land, and
fb0's matmuls run tb-major with i-pair PSUM groups so mm1 starts once
the first 4 rows have landed, with rows 4-7 transposed in the gaps.

mm2 runs f-outer / tt-inner with all 8 PSUM banks accumulating; W2
stripes trickle in ~12 f-tiles ahead of consumption.  Output drains
alternate the Scalar and Vector engines (copy + per-engine DMA queue),
and the last h-block's f-tail is processed per-tt so the drain
staggers into a short tail.
"""

from contextlib import ExitStack

import numpy as np

import concourse.bass as bass
import concourse.mybir as mybir
import concourse.tile as tile
from concourse import bacc
from concourse.bass_utils import run_bass_kernel_spmd
from concourse.masks import make_identity

N_CORES = 8
P = 128
TB = 512  # moving-operand free-dim block (one PSUM bank of f32)

F32 = mybir.dt.float32
BF16 = mybir.dt.bfloat16
ACT_SILU = mybir.ActivationFunctionType.Silu
ACT_COPY = mybir.ActivationFunctionType.Copy

# Per-core problem dims (full problem: 8 experts x 1024 tokens, H=2048, F=4096)
T = 1024
H = 2048
F = 4096


def build_kernel_body(tc, T=T, H=H, F=F):
    nc = tc.nc
    h_d = nc.dram_tensor("hidden_states", [T, H], F32, kind="ExternalInput").ap()
    w1_d = nc.dram_tensor("gate_up_proj", [H, 2 * F], F32, kind="ExternalInput").ap()
    w2_d = nc.dram_tensor("down_proj", [F, H], F32, kind="ExternalInput").ap()
    out_d = nc.dram_tensor("out", [T, H], F32, kind="ExternalOutput").ap()

    n_ht = H // P          # h-tiles (contraction tiles of matmul 1)
    n_ft = F // P          # f-tiles (rows of act; contraction tiles of matmul 2)
    n_tt = T // P          # token tiles (psum partition tiles of matmul 2)
    n_tb = T // TB         # token free-dim blocks in matmul 1
    n_fb = F // TB         # 512-wide f blocks of W1 (per gate/up half)
    n_hb = H // TB         # 512-wide h blocks of W2

    with ExitStack() as ctx:
        const = ctx.enter_context(tc.tile_pool(name="const", bufs=1))
        hbfp = ctx.enter_context(tc.tile_pool(name="hbfp", bufs=4))
        htp = ctx.enter_context(tc.tile_pool(name="htp", bufs=n_ht))
        actp = ctx.enter_context(tc.tile_pool(name="actp", bufs=n_ft))
        w1p = ctx.enter_context(tc.tile_pool(name="w1p", bufs=62))
        b2p = ctx.enter_context(tc.tile_pool(name="b2p", bufs=12))
        silp = ctx.enter_context(tc.tile_pool(name="silp", bufs=2))
        outp = ctx.enter_context(tc.tile_pool(name="outp", bufs=8))
        ps = ctx.enter_context(tc.tile_pool(name="ps", bufs=8, space="PSUM"))

        identb = const.tile([P, P], BF16, tag="identb", name="identb")
        make_identity(nc, identb)

        ht = [htp.tile([P, T], BF16, tag="ht", name=f"ht{i}") for i in range(n_ht)]
        act = [actp.tile([P, T], BF16, tag="act", name=f"act{i}") for i in range(n_ft)]

        # ---- h rows: gpsimd casting DMAs (f32 HBM -> bf16 SBUF) ----
        hrow = {}

        def dma_h_row(ti):
            hr = hbfp.tile([P, H], BF16, tag="hrow", name=f"hrow{ti}")
            nc.gpsimd.dma_start(out=hr[:], in_=h_d[ti * P : (ti + 1) * P, :])
            hrow[ti] = hr

        def transpose_row(ti):
            """PE-transpose one bf16 128-token row block into ht.

            bf16 transposes run at 1 PE cycle/row (f32 takes 2 via the
            LOW/HIGH passes); the psum bank is viewed as bf16 via bitcast.
            """
            hr = hrow[ti]
            for hh in range(n_ht):
                pt = ps.tile([P, TB], F32, tag="ps", name=f"tp{ti}_{hh}")
                ptb = pt[:].bitcast(BF16)
                nc.tensor.transpose(
                    ptb[:, :P], hr[:, hh * P : (hh + 1) * P], identb
                )
                nc.vector.tensor_copy(
                    out=ht[hh][:, ti * P : (ti + 1) * P], in_=ptb[:, :P]
                )

        # ---- W1: gpsimd casting DMAs, f32 HBM -> bf16 SBUF stripes ----
        w1g, w1u = {}, {}

        def dma_w1_block(fb):
            c0 = fb * TB
            for hh in range(n_ht):
                g = w1p.tile([P, TB], BF16, tag="w1", name=f"w1g_{fb}_{hh}")
                nc.gpsimd.dma_start(
                    out=g[:], in_=w1_d[hh * P : (hh + 1) * P, c0 : c0 + TB]
                )
                u = w1p.tile([P, TB], BF16, tag="w1", name=f"w1u_{fb}_{hh}")
                nc.gpsimd.dma_start(
                    out=u[:], in_=w1_d[hh * P : (hh + 1) * P, F + c0 : F + c0 + TB]
                )
                w1g[(fb, hh)] = g
                w1u[(fb, hh)] = u

        def swiglu(fi, tb, pg, pu):
            sg = silp.tile([P, TB], BF16, tag="silp", name=f"sig{fi}_{tb}")
            nc.scalar.activation(sg[:], pg[:], ACT_SILU)
            nc.vector.tensor_mul(
                out=act[fi][:, tb * TB : (tb + 1) * TB], in0=pu[:], in1=sg[:]
            )

        # ---- W2: gpsimd casting DMAs straight into the b2 ring ----
        b2 = {}

        def w2_stripe(hb, f):
            b = b2p.tile([P, TB], BF16, tag="b2", name=f"b2_{hb}_{f}")
            nc.gpsimd.dma_start(
                out=b[:], in_=w2_d[f * P : (f + 1) * P, hb * TB : (hb + 1) * TB]
            )
            b2[(hb, f)] = b

        # ---- Phase A/B0: h loads + transposes interleaved with fb0 ----
        # gpsimd issue order: rows 0-3, all of fb0, rows 4-7 (the late rows
        # WAR-wait on rows 0-3's transposes, so they must trail fb0 to
        # avoid head-of-line blocking the W1 stream).
        for ti in range(4):
            dma_h_row(ti)
        dma_w1_block(0)
        for ti in range(4, 8):
            dma_h_row(ti)
        for ti in range(4):
            transpose_row(ti)

        def sweep_fb0(tb, iset):
            """h-outer sweep over one tb-half of fb0 for an i-pair.

            Paces off the W1 DMA stream: matmul (i, hh) waits only on the
            gpsimd casting DMA of stripe (0, hh).
            """
            pg = {i: ps.tile([P, TB], F32, tag="ps", name=f"pg0_{i}_{tb}")
                  for i in iset}
            pu = {i: ps.tile([P, TB], F32, tag="ps", name=f"pu0_{i}_{tb}")
                  for i in iset}
            for hh in range(n_ht):
                first, last = hh == 0, hh == n_ht - 1
                for i in iset:
                    for p, w in ((pg, w1g), (pu, w1u)):
                        nc.tensor.matmul(
                            p[i][:],
                            lhsT=w[(0, hh)][:, i * P : (i + 1) * P],
                            rhs=ht[hh][:, tb * TB : (tb + 1) * TB],
                            start=first,
                            stop=last,
                        )
            for i in iset:
                swiglu(i, tb, pg[i], pu[i])

        # rows 4-7 transpose between sweeps (they land while sweep A runs;
        # no psum accumulation groups are held at these points).  fb1's
        # stripes are issued after sweep A so startup DMA bandwidth goes
        # to h and fb0 first.
        sweep_fb0(0, (0, 1))
        dma_w1_block(1)
        transpose_row(4)
        transpose_row(5)
        sweep_fb0(0, (2, 3))
        transpose_row(6)
        transpose_row(7)
        sweep_fb0(1, (0, 1))
        sweep_fb0(1, (2, 3))

        # ---- Phase B: steady-state f-blocks 1..n_fb-1 ----
        for fb in range(1, n_fb):
            if fb + 1 < n_fb:
                dma_w1_block(fb + 1)
            for i in range(TB // P):
                fi = fb * (TB // P) + i
                pg = {tb: ps.tile([P, TB], F32, tag="ps", name=f"pg{fi}_{tb}")
                      for tb in range(n_tb)}
                pu = {tb: ps.tile([P, TB], F32, tag="ps", name=f"pu{fi}_{tb}")
                      for tb in range(n_tb)}
                for hh in range(n_ht):
                    first, last = hh == 0, hh == n_ht - 1
                    for p, w in ((pg, w1g), (pu, w1u)):
                        for tb in range(n_tb):
                            nc.tensor.matmul(
                                p[tb][:],
                                lhsT=w[(fb, hh)][:, i * P : (i + 1) * P],
                                rhs=ht[hh][:, tb * TB : (tb + 1) * TB],
                                start=first,
                                stop=last,
                            )
                for tb in range(n_tb):
                    swiglu(fi, tb, pg[tb], pu[tb])
            if fb == n_fb - 2:
                # W2 hb0 prefetch (capped at the b2 ring depth)
                for f in range(12):
                    w2_stripe(0, f)

        # ---- Phase C: out = act @ W2, f-outer / tt-inner, 8 psum banks ----
        def drain(hb, tt, po):
            ob = outp.tile([P, TB], F32, tag="outp", name=f"ob{hb}_{tt}")
            if tt % 2 == 0:
                nc.scalar.activation(ob[:], po[:], ACT_COPY)
                nc.scalar.dma_start(
                    out_d[tt * P : (tt + 1) * P, hb * TB : (hb + 1) * TB], ob[:]
                )
            else:
                nc.vector.tensor_copy(out=ob[:], in_=po[:])
                nc.sync.dma_start(
                    out_d[tt * P : (tt + 1) * P, hb * TB : (hb + 1) * TB], ob[:]
                )

        # strict FIFO stripe schedule: allocation runs exactly AHEAD of
        # consumption so the b2 ring can never form a cross-phase cycle
        SCHED = [(hb, f) for hb in range(n_hb) for f in range(n_ft)]
        AHEAD = 12

        def trickle(g):
            if g + AHEAD < len(SCHED):
                w2_stripe(*SCHED[g + AHEAD])

        FTAIL = 4  # last-hb f-tiles processed per-tt so the drain staggers
        for hb in range(n_hb):
            last_hb = hb == n_hb - 1
            po = [ps.tile([P, TB], F32, tag="ps", name=f"po{hb}_{tt}")
                  for tt in range(n_tt)]
            n_f_main = n_ft - FTAIL if last_hb else n_ft
            for f in range(n_f_main):
                trickle(hb * n_ft + f)
                rhs = b2[(hb, f)][:]
                for tt in range(n_tt):
                    nc.tensor.matmul(
                        po[tt][:],
                        lhsT=act[f][:, tt * P : (tt + 1) * P],
                        rhs=rhs,
                        start=(f == 0),
                        stop=(not last_hb and f == n_ft - 1),
                    )
            if not last_hb:
                for tt in range(n_tt):
                    drain(hb, tt, po[tt])
            else:
                for f in range(n_f_main, n_ft):
                    trickle(hb * n_ft + f)
                for tt in range(n_tt):
                    for f in range(n_f_main, n_ft):
                        nc.tensor.matmul(
                            po[tt][:],
                            lhsT=act[f][:, tt * P : (tt + 1) * P],
                            rhs=b2[(hb, f)][:],
                            start=False,
                            stop=(f == n_ft - 1),
                        )
                    if tt < n_tt - 2:
                        drain(hb, tt, po[tt])
                    else:
                        # final drains split in half across scalar+vector so
                        # the copy and out-DMA tail overlap
                        ob = outp.tile([P, TB], F32, tag="outp",
                                       name=f"ob{hb}_{tt}")
                        hw = TB // 2
                        nc.scalar.activation(ob[:, :hw], po[tt][:, :hw],
                                             ACT_COPY)
                        nc.scalar.dma_start(
                            out_d[tt * P : (tt + 1) * P,
                                  hb * TB : hb * TB + hw],
                            ob[:, :hw],
                        )
                        nc.vector.tensor_copy(out=ob[:, hw:],
                                              in_=po[tt][:, hw:])
                        nc.sync.dma_start(
                            out_d[tt * P : (tt + 1) * P,
                                  hb * TB + hw : (hb + 1) * TB],
                            ob[:, hw:],
                        )


def build_nc(T=T, H=H, F=F):
    nc = bacc.Bacc(
        "TRN2", target_bir_lowering=False, debug=False, enable_asserts=False
    )
    with tile.TileContext(nc) as tc:
        build_kernel_body(tc, T=T, H=H, F=F)
    nc.compile()
    return nc


_NC_CACHE = None


def run(hidden_states, gate_up_proj, down_proj, trace=False, **kw):
    """Run on the 8 NeuronCores; returns (output, BassKernelResults)."""
    global _NC_CACHE
    if _NC_CACHE is None:
        _NC_CACHE = build_nc()
    nc = _NC_CACHE

    hs = np.ascontiguousarray(np.asarray(hidden_states), dtype=np.float32)
    gup = np.ascontiguousarray(np.asarray(gate_up_proj), dtype=np.float32)
    dp = np.ascontiguousarray(np.asarray(down_proj), dtype=np.float32)
    assert hs.shape == (N_CORES * T, H), hs.shape
    assert gup.shape == (N_CORES, H, 2 * F), gup.shape
    assert dp.shape == (N_CORES, F, H), dp.shape

    in_maps = [
        {
            "hidden_states": np.ascontiguousarray(hs[i * T : (i + 1) * T]),
            "gate_up_proj": np.ascontiguousarray(gup[i]),
            "down_proj": np.ascontiguousarray(dp[i]),
        }
        for i in range(N_CORES)
    ]
    res = run_bass_kernel_spmd(
        nc, in_maps, core_ids=list(range(N_CORES)), trace=trace, **kw
    )
    out = np.concatenate(
        [res.results[i]["out"] for i in range(N_CORES)], axis=0
    ).astype(np.float32)
    return out, res


def kernel(hidden_states, gate_up_proj, down_proj):
    out, _ = run(hidden_states, gate_up_proj, down_proj, trace=False)
    return out
